# revision 24
# baseline (speedup 1.0000x reference)
"""Multi-head causal attention (B=4, L=2048, D=1024, H=16) on 8 TRN2 NeuronCores.

Sharding: core c handles batch b = c//2 and head-group hg = c%2 (8 heads, 512
dims). Each core computes Q/K/V projections for its heads, causal attention,
and a partial output projection (its 512 input dims of Wo). Host sums the two
partials per batch (fp16 partials, fp32 sum).

Host pre-transposes and pre-casts all inputs to fp16 (x^T, Wq^T*0.125, Wk^T,
Wv^T, Wo^T), so the device does no transposes and DMA-in is halved.

AV runs with V (+ones column) stationary and P^T moving (N=512 streams),
producing out^T in PSUM directly in the layout the output projection needs.
The ones column makes row 64 the softmax denominator; normalization is
reciprocal + a K=1 fp32r broadcast matmul + one fused multiply-cast.
Scores/exp/AV streams are causally trimmed; a single [128,128] triangle mask
zeroes the diagonal blocks.
"""
import sys

sys.path.insert(0, "/opt/trn_rl_repo")

import numpy as np

import concourse.bass as bass
import concourse.mybir as mybir
import concourse.tile as tile
from concourse import bacc

F32 = mybir.dt.float32
F32R = mybir.dt.float32r
F16 = mybir.dt.float16
MM = F16
AF = mybir.ActivationFunctionType

B, L, D, H = 4, 2048, 1024, 16
DK = 64
E = 512
ND = D // 128   # 8 d-tiles
NE = E // 128   # 4 e-tiles
NL = L // 128   # 16 l-tiles
NJ = L // 512   # 4 q-chunks
NK = L // 128   # 16 k-tiles

_CACHE = {}


def build_program():
    nc = bacc.Bacc("TRN2", target_bir_lowering=False, debug=False, num_devices=8)

    xTd = nc.dram_tensor("xT", [D, L], F16, kind="ExternalInput")
    wqT = nc.dram_tensor("wqT", [D, E], F16, kind="ExternalInput")
    wkT = nc.dram_tensor("wkT", [D, E], F16, kind="ExternalInput")
    wvT = nc.dram_tensor("wvT", [D, E], F16, kind="ExternalInput")
    woT = nc.dram_tensor("woT", [E, D], F16, kind="ExternalInput")
    trid = nc.dram_tensor("tri", [128, 128], F16, kind="ExternalInput")
    out = nc.dram_tensor("out", [L, D], F16, kind="ExternalOutput")

    with tile.TileContext(nc) as tc:
        with (
            tc.tile_pool(name="const", bufs=1) as constp,
            tc.tile_pool(name="big", bufs=1) as bigp,
            tc.tile_pool(name="qtc", bufs=2) as qtcp,
            tc.tile_pool(name="ptp", bufs=2) as ptp,
            tc.tile_pool(name="smallp", bufs=2) as smallp,
            tc.tile_pool(name="psS", bufs=2, space="PSUM") as psS,
            tc.tile_pool(name="psP", bufs=2, space="PSUM") as psP,
            tc.tile_pool(name="psA", bufs=2, space="PSUM") as psA,
        ):
            warm_sb = constp.tile([128, 512], MM)
            nc.gpsimd.memset(warm_sb[:], 0.03125)
            tri_sb = constp.tile([128, 128], F16)

            xT = bigp.tile([128, ND, L], MM)    # x^T  [d-in-tile, d-tile, l]
            WTq = bigp.tile([128, ND, E], MM)   # Wq^T [d-in-tile, d-tile, e]
            WTk = bigp.tile([128, ND, E], MM)
            WTv = bigp.tile([128, ND, E], MM)
            KT = bigp.tile([128, NE, L], MM)    # K^T  [dk (2 heads), e-tile, k]
            WoT = bigp.tile([128, NE, D], MM)   # Wo^T [e-in-tile, e-tile, d]
            attT = bigp.tile([128, NE, L], MM)  # att^T [dk (2 heads), e-tile, l]
            # V per (k-tile, head); cols 64:128 are ones so the AV matmul
            # replicates the softmax denominator across psum partitions 64:128
            Vaug = bigp.tile([128, NK, 8, 128], MM)

            # ---------- input DMAs (issued up-front, priority order) ----------
            nc.sync.dma_start(tri_sb[:], trid[:, :])
            nc.sync.dma_start(
                WTk[:, :, :], wkT[:, :].rearrange("(dt p) e -> p dt e", p=128)
            )
            for dt in range(ND):
                nc.sync.dma_start(xT[:, dt, 0:512], xTd[dt * 128:(dt + 1) * 128, 0:512])
            for wsb, wdr in ((WTv, wvT), (WTq, wqT)):
                nc.sync.dma_start(
                    wsb[:, :, :],
                    wdr[:, :].rearrange("(dt p) e -> p dt e", p=128),
                )
            nc.sync.dma_start(
                xT[:, :, 512:L],
                xTd[:, 512:L].rearrange("(dt p) l -> p dt l", p=128),
            )
            nc.sync.dma_start(
                WoT[:, :, :],
                woT[:, :].rearrange("(et p) d -> p et d", p=128),
            )

            # ones columns of Vaug (denominator trick, replicated 64x)
            nc.vector.memset(Vaug[:, :, :, 64:128], 1.0)

            # ---------- PE warmup (un-throttle the HAM before real work) ----
            wps0 = None
            for i in range(16):
                wps = psP.tile([128, 512], F32, tag="pp", name="wps")
                nc.tensor.matmul(wps[:], warm_sb[:, 0:128], warm_sb[:])
                if i == 0:
                    wps0 = wps
            # preload the Exp act table during the DMA shadow
            scr = smallp.tile([128, 16], F16, tag="scr", name="scr")
            nc.scalar.activation(scr[:], wps0[:, 0:16], AF.Exp)

            # ---------- emission helpers ----------
            def k_proj_group(et, jc):
                def gen():
                    pp = psP.tile([128, 512], F32, tag="pp", name="pp")
                    for dt in range(ND):
                        yield lambda dt=dt, pp=pp: nc.tensor.matmul(
                            pp[:],
                            WTk[:, dt, et * 128:(et + 1) * 128],
                            xT[:, dt, jc * 512:(jc + 1) * 512],
                            start=(dt == 0),
                            stop=(dt == ND - 1),
                        )
                    yield lambda pp=pp: nc.vector.tensor_copy(
                        KT[:, et, jc * 512:(jc + 1) * 512], pp[:]
                    )
                return gen()

            def v_proj_group(lt):
                def gen():
                    pp = psP.tile([128, 512], F32, tag="pp", name="pp")
                    for dt in range(ND):
                        yield lambda dt=dt, pp=pp: nc.tensor.matmul(
                            pp[:],
                            xT[:, dt, lt * 128:(lt + 1) * 128],
                            WTv[:, dt, :],
                            start=(dt == 0),
                            stop=(dt == ND - 1),
                        )
                    yield lambda pp=pp: nc.vector.tensor_copy(
                        Vaug[:, lt, :, 0:64], pp[:]
                    )
                return gen()

            def q_proj_group(j, qtile, et):
                def gen():
                    pp = psP.tile([128, 512], F32, tag="pp", name="pp")
                    for dt in range(ND):
                        yield lambda dt=dt, pp=pp: nc.tensor.matmul(
                            pp[:],
                            WTq[:, dt, et * 128:(et + 1) * 128],
                            xT[:, dt, j * 512:(j + 1) * 512],
                            start=(dt == 0),
                            stop=(dt == ND - 1),
                        )
                    yield lambda pp=pp: nc.vector.tensor_copy(qtile[:, et, :], pp[:])
                return gen()

            def out_proj_group(lt, ec):
                def gen():
                    op = psP.tile([128, 512], F32, tag="pp", name="op")
                    for dt in range(NE):
                        yield lambda dt=dt, op=op: nc.tensor.matmul(
                            op[:],
                            attT[:, dt, lt * 128:(lt + 1) * 128],
                            WoT[:, dt, ec * 512:(ec + 1) * 512],
                            start=(dt == 0),
                            stop=(dt == NE - 1),
                        )
                    def tail(op=op):
                        ot = smallp.tile([128, 512], F16, tag="ot", name="ot")
                        nc.vector.tensor_copy(ot[:], op[:])
                        nc.sync.dma_start(
                            out[lt * 128:(lt + 1) * 128, ec * 512:(ec + 1) * 512],
                            ot[:],
                        )
                    yield tail
                return gen()

            def chain(gens):
                for g in gens:
                    yield from g

            def drain(it, n):
                k = 0
                for f in it:
                    f()
                    k += 1
                    if k >= n:
                        return

            # ---------- prologue: chunk-0 projections ----------
            qtiles = {0: qtcp.tile([128, NE, 512], MM, tag="qt", name="qt0")}
            for et in range(NE):
                drain(k_proj_group(et, 0), 99)
            for lt in range(4):
                drain(v_proj_group(lt), 99)
            for et in range(NE):
                drain(q_proj_group(0, qtiles[0], et), 99)

            # ---------- chunk loop ----------
            pending_norm = None

            def emit_norm(pn):
                avT, rc, h, jj = pn
                hb = h >> 1
                if h & 1:
                    tmp = smallp.tile([64, 512], F16, tag="nt", name="nt")
                    nc.vector.tensor_mul(tmp[:], avT[0:64, :], rc[:])
                    nc.sync.dma_start(
                        attT[64:128, hb, jj * 512:(jj + 1) * 512], tmp[:]
                    )
                else:
                    nc.vector.tensor_mul(
                        attT[0:64, hb, jj * 512:(jj + 1) * 512],
                        avT[0:64, :],
                        rc[:],
                    )

            for j in range(NJ):
                nkt = 4 * (j + 1)
                npairs = nkt // 2
                gens = []
                n_items = 0
                if j + 1 < NJ:
                    qtiles[j + 1] = qtcp.tile(
                        [128, NE, 512], MM, tag="qt", name=f"qt{j+1}"
                    )
                    for et in range(NE):
                        gens.append(k_proj_group(et, j + 1))
                    for lt in range(4 * (j + 1), 4 * (j + 1) + 4):
                        gens.append(v_proj_group(lt))
                    for et in range(NE):
                        gens.append(q_proj_group(j + 1, qtiles[j + 1], et))
                    n_items += 12 * 9
                if j >= 1:
                    for lt in range(4 * (j - 1), 4 * (j - 1) + 4):
                        for ec in range(2):
                            gens.append(out_proj_group(lt, ec))
                    n_items += 8 * 6
                filler = chain(gens)
                steps = 8 * npairs
                per_step = n_items / steps if steps else 0.0
                QTc = qtiles[j]
                fill_acc = 0.0
                fill_done = 0

                for h in range(8):
                    hb, hp = h >> 1, (h & 1) * 64
                    PTt = ptp.tile([128, NK, 512], MM, tag="pt", name="pt")
                    avT = psA.tile([128, 512], F32, tag="av", name="avT")
                    for p in range(npairs):
                        sps = psS.tile([128, 2, 512], F32, tag="s", name="sps")
                        for half in (0, 1):
                            kt = 2 * p + half
                            r = kt - 4 * j
                            c0 = max(0, r * 128)
                            nc.tensor.matmul(
                                sps[:, half, c0:512],
                                KT[hp:hp + 64, hb, kt * 128:(kt + 1) * 128],
                                QTc[hp:hp + 64, hb, c0:512],
                            )
                        if 2 * p >= 4 * j:
                            # both k-tiles of this pair are diagonal: trim exp
                            for half in (0, 1):
                                kt = 2 * p + half
                                c0 = (kt - 4 * j) * 128
                                nc.scalar.activation(
                                    PTt[:, kt, c0:512], sps[:, half, c0:512], AF.Exp
                                )
                        else:
                            nc.scalar.activation(
                                PTt[:, 2 * p:2 * p + 2, :], sps[:], AF.Exp
                            )
                        for half in (0, 1):
                            kt = 2 * p + half
                            r = kt - 4 * j
                            if r >= 0:
                                nc.vector.tensor_mul(
                                    PTt[:, kt, r * 128:(r + 1) * 128],
                                    PTt[:, kt, r * 128:(r + 1) * 128],
                                    tri_sb[:],
                                )
                        if p == 0 and pending_norm is not None:
                            emit_norm(pending_norm)
                            pending_norm = None
                        if p > 0:
                            for kt in (2 * p - 2, 2 * p - 1):
                                r = kt - 4 * j
                                c0 = max(0, r * 128)
                                nc.tensor.matmul(
                                    avT[:, c0:512],
                                    Vaug[:, kt, h, :],
                                    PTt[:, kt, c0:512],
                                    start=(kt == 0),
                                    stop=(kt == nkt - 1),
                                )
                        fill_acc += per_step
                        take = int(fill_acc) - fill_done
                        if take > 0:
                            drain(filler, take)
                            fill_done += take
                    for kt in (nkt - 2, nkt - 1):
                        r = kt - 4 * j
                        c0 = max(0, r * 128)
                        nc.tensor.matmul(
                            avT[:, c0:512],
                            Vaug[:, kt, h, :],
                            PTt[:, kt, c0:512],
                            start=(kt == 0),
                            stop=(kt == nkt - 1),
                        )
                    # DVE: cross-base copy to SBUF + fast approx reciprocal
                    dcp = smallp.tile([64, 512], F32, tag="dc", name="dcp")
                    nc.vector.tensor_copy(dcp[:], avT[64:128, :])
                    rc = smallp.tile([64, 512], F32, tag="rca", name="rca")
                    nc.vector.reciprocal_approx_fast(rc[:], dcp[:])
                    pending_norm = (avT, rc, h, j)
                if j == NJ - 1 and pending_norm is not None:
                    emit_norm(pending_norm)
                    pending_norm = None
                drain(filler, 10 ** 9)
                if pending_norm is not None:
                    emit_norm(pending_norm)
                    pending_norm = None

            for lt in range(4 * (NJ - 1), 4 * (NJ - 1) + 4):
                for ec in range(2):
                    drain(out_proj_group(lt, ec), 99)

    nc.compile()
    return nc


def build_tri():
    kp = np.arange(128)[:, None]
    qf = np.arange(128)[None, :]
    return (qf >= kp).astype(np.float16)


def _get_program():
    if "nc" not in _CACHE:
        _CACHE["nc"] = build_program()
    return _CACHE["nc"]


def make_in_maps(x, Wq, Wk, Wv, Wo):
    x = np.asarray(x, dtype=np.float32)
    Wq = np.asarray(Wq, dtype=np.float32)
    Wk = np.asarray(Wk, dtype=np.float32)
    Wv = np.asarray(Wv, dtype=np.float32)
    Wo = np.asarray(Wo, dtype=np.float32)
    tri = build_tri()
    xTs = [np.ascontiguousarray(x[b].T).astype(np.float16) for b in range(B)]
    in_maps = []
    for c in range(8):
        b, hg = c // 2, c % 2
        sl = slice(hg * E, (hg + 1) * E)
        in_maps.append(
            {
                "xT": xTs[b],
                "wqT": np.ascontiguousarray((Wq[sl] * 0.125).T).astype(np.float16),
                "wkT": np.ascontiguousarray(Wk[sl].T).astype(np.float16),
                "wvT": np.ascontiguousarray(Wv[sl].T).astype(np.float16),
                "woT": np.ascontiguousarray(Wo[:, sl].T).astype(np.float16),
                "tri": tri,
            }
        )
    return in_maps


def kernel(x, Wq, Wk, Wv, Wo, **run_kwargs):
    from concourse import bass_utils

    nc = _get_program()
    in_maps = make_in_maps(x, Wq, Wk, Wv, Wo)
    res = bass_utils.run_bass_kernel_spmd(
        nc, in_maps, core_ids=list(range(8)), **run_kwargs
    )
    o = np.empty((B, L, D), np.float32)
    for b in range(B):
        o[b] = res.results[2 * b]["out"].astype(np.float32) + res.results[
            2 * b + 1
        ]["out"].astype(np.float32)
    _CACHE["last_result"] = res
    return o


# revision 27
# speedup vs baseline: 1.0069x; 1.0069x over previous
"""Multi-head causal attention (B=4, L=2048, D=1024, H=16) on 8 TRN2 NeuronCores.

Sharding: core c handles batch b = c//2 and head-group hg = c%2 (8 heads, 512
dims). Each core computes Q/K/V projections for its heads, causal attention,
and a partial output projection (its 512 input dims of Wo). Host sums the two
partials per batch (fp16 partials, fp32 sum).

Host pre-transposes and pre-casts all inputs to fp16 (x^T, Wq^T*0.125, Wk^T,
Wv^T, Wo^T), so the device does no transposes and DMA-in is halved.

AV runs with V (+ones column) stationary and P^T moving (N=512 streams),
producing out^T in PSUM directly in the layout the output projection needs.
The ones column makes row 64 the softmax denominator; normalization is
reciprocal + a K=1 fp32r broadcast matmul + one fused multiply-cast.
Scores/exp/AV streams are causally trimmed; a single [128,128] triangle mask
zeroes the diagonal blocks.
"""
import sys

sys.path.insert(0, "/opt/trn_rl_repo")

import numpy as np

import concourse.bass as bass
import concourse.mybir as mybir
import concourse.tile as tile
from concourse import bacc

F32 = mybir.dt.float32
F32R = mybir.dt.float32r
F16 = mybir.dt.float16
MM = F16
AF = mybir.ActivationFunctionType

B, L, D, H = 4, 2048, 1024, 16
DK = 64
E = 512
ND = D // 128   # 8 d-tiles
NE = E // 128   # 4 e-tiles
NL = L // 128   # 16 l-tiles
NJ = L // 512   # 4 q-chunks
NK = L // 128   # 16 k-tiles

_CACHE = {}


def build_program():
    nc = bacc.Bacc("TRN2", target_bir_lowering=False, debug=False, num_devices=8)

    xTd = nc.dram_tensor("xT", [D, L], F16, kind="ExternalInput")
    wqT = nc.dram_tensor("wqT", [D, E], F16, kind="ExternalInput")
    wkT = nc.dram_tensor("wkT", [D, E], F16, kind="ExternalInput")
    wvT = nc.dram_tensor("wvT", [D, E], F16, kind="ExternalInput")
    woT = nc.dram_tensor("woT", [E, D], F16, kind="ExternalInput")
    trid = nc.dram_tensor("tri", [128, 128], F16, kind="ExternalInput")
    out = nc.dram_tensor("out", [L, D], F16, kind="ExternalOutput")

    with tile.TileContext(nc) as tc:
        with (
            tc.tile_pool(name="const", bufs=1) as constp,
            tc.tile_pool(name="big", bufs=1) as bigp,
            tc.tile_pool(name="qtc", bufs=2) as qtcp,
            tc.tile_pool(name="ptp", bufs=2) as ptp,
            tc.tile_pool(name="smallp", bufs=2) as smallp,
            tc.tile_pool(name="psS", bufs=2, space="PSUM") as psS,
            tc.tile_pool(name="psP", bufs=2, space="PSUM") as psP,
            tc.tile_pool(name="psA", bufs=2, space="PSUM") as psA,
        ):
            warm_sb = constp.tile([128, 512], MM)
            nc.gpsimd.memset(warm_sb[:], 0.03125)
            tri_sb = constp.tile([128, 128], F16)

            xT = bigp.tile([128, ND, L], MM)    # x^T  [d-in-tile, d-tile, l]
            WTq = bigp.tile([128, ND, E], MM)   # Wq^T [d-in-tile, d-tile, e]
            WTk = bigp.tile([128, ND, E], MM)
            WTv = bigp.tile([128, ND, E], MM)
            KT = bigp.tile([128, NE, L], MM)    # K^T  [dk (2 heads), e-tile, k]
            WoT = bigp.tile([128, NE, D], MM)   # Wo^T [e-in-tile, e-tile, d]
            attT = bigp.tile([128, NE, L], MM)  # att^T [dk (2 heads), e-tile, l]
            # V per (k-tile, head); cols 64:128 are ones so the AV matmul
            # replicates the softmax denominator across psum partitions 64:128
            Vaug = bigp.tile([128, NK, 8, 128], MM)

            # ---------- input DMAs (issued up-front, priority order) ----------
            nc.sync.dma_start(
                WTk[:, :, :], wkT[:, :].rearrange("(dt p) e -> p dt e", p=128)
            )
            nc.sync.dma_start(
                xT[:, :, 0:512],
                xTd[:, 0:512].rearrange("(dt p) l -> p dt l", p=128),
            )
            for wsb, wdr in ((WTv, wvT), (WTq, wqT)):
                nc.sync.dma_start(
                    wsb[:, :, :],
                    wdr[:, :].rearrange("(dt p) e -> p dt e", p=128),
                )
            nc.sync.dma_start(tri_sb[:], trid[:, :])
            nc.sync.dma_start(
                xT[:, :, 512:L],
                xTd[:, 512:L].rearrange("(dt p) l -> p dt l", p=128),
            )
            nc.sync.dma_start(
                WoT[:, :, :],
                woT[:, :].rearrange("(et p) d -> p et d", p=128),
            )

            # ones columns of Vaug (denominator trick, replicated 64x)
            nc.vector.memset(Vaug[:, :, :, 64:128], 1.0)

            # ---------- PE warmup (un-throttle the HAM before real work) ----
            wps = psP.tile([128, 512], F32, tag="pp", name="wps")
            for _ in range(16):
                nc.tensor.matmul(wps[:], warm_sb[:, 0:128], warm_sb[:])
            # preload the Exp act table during the DMA shadow
            scr = smallp.tile([128, 16], F16, tag="scr", name="scr")
            nc.scalar.activation(scr[:], wps[:, 0:16], AF.Exp)

            # ---------- emission helpers ----------
            def k_proj_group(et, jc):
                def gen():
                    pp = psP.tile([128, 512], F32, tag="pp", name="pp")
                    for dt in range(ND):
                        yield lambda dt=dt, pp=pp: nc.tensor.matmul(
                            pp[:],
                            WTk[:, dt, et * 128:(et + 1) * 128],
                            xT[:, dt, jc * 512:(jc + 1) * 512],
                            start=(dt == 0),
                            stop=(dt == ND - 1),
                        )
                    yield lambda pp=pp: nc.vector.tensor_copy(
                        KT[:, et, jc * 512:(jc + 1) * 512], pp[:]
                    )
                return gen()

            def v_proj_group(lt):
                def gen():
                    pp = psP.tile([128, 512], F32, tag="pp", name="pp")
                    for dt in range(ND):
                        yield lambda dt=dt, pp=pp: nc.tensor.matmul(
                            pp[:],
                            xT[:, dt, lt * 128:(lt + 1) * 128],
                            WTv[:, dt, :],
                            start=(dt == 0),
                            stop=(dt == ND - 1),
                        )
                    yield lambda pp=pp: nc.vector.tensor_copy(
                        Vaug[:, lt, :, 0:64], pp[:]
                    )
                return gen()

            def q_proj_group(j, qtile, et):
                def gen():
                    pp = psP.tile([128, 512], F32, tag="pp", name="pp")
                    for dt in range(ND):
                        yield lambda dt=dt, pp=pp: nc.tensor.matmul(
                            pp[:],
                            WTq[:, dt, et * 128:(et + 1) * 128],
                            xT[:, dt, j * 512:(j + 1) * 512],
                            start=(dt == 0),
                            stop=(dt == ND - 1),
                        )
                    yield lambda pp=pp: nc.vector.tensor_copy(qtile[:, et, :], pp[:])
                return gen()

            def out_proj_group(lt, ec):
                def gen():
                    op = psP.tile([128, 512], F32, tag="pp", name="op")
                    for dt in range(NE):
                        yield lambda dt=dt, op=op: nc.tensor.matmul(
                            op[:],
                            attT[:, dt, lt * 128:(lt + 1) * 128],
                            WoT[:, dt, ec * 512:(ec + 1) * 512],
                            start=(dt == 0),
                            stop=(dt == NE - 1),
                        )
                    def tail(op=op):
                        ot = smallp.tile([128, 512], F16, tag="ot", name="ot")
                        nc.vector.tensor_copy(ot[:], op[:])
                        nc.sync.dma_start(
                            out[lt * 128:(lt + 1) * 128, ec * 512:(ec + 1) * 512],
                            ot[:],
                        )
                    yield tail
                return gen()

            def chain(gens):
                for g in gens:
                    yield from g

            def drain(it, n):
                k = 0
                for f in it:
                    f()
                    k += 1
                    if k >= n:
                        return

            # ---------- prologue: chunk-0 projections ----------
            qtiles = {0: qtcp.tile([128, NE, 512], MM, tag="qt", name="qt0")}
            for et in range(NE):
                drain(k_proj_group(et, 0), 99)
            for lt in range(4):
                drain(v_proj_group(lt), 99)
            for et in range(NE):
                drain(q_proj_group(0, qtiles[0], et), 99)

            # ---------- chunk loop ----------
            pending_norm = None

            def emit_norm(pn):
                avT, rc, h, jj = pn
                hb = h >> 1
                if h & 1:
                    tmp = smallp.tile([64, 512], F16, tag="nt", name="nt")
                    nc.vector.tensor_mul(tmp[:], avT[0:64, :], rc[:])
                    nc.sync.dma_start(
                        attT[64:128, hb, jj * 512:(jj + 1) * 512], tmp[:]
                    )
                else:
                    nc.vector.tensor_mul(
                        attT[0:64, hb, jj * 512:(jj + 1) * 512],
                        avT[0:64, :],
                        rc[:],
                    )

            for j in range(NJ):
                nkt = 4 * (j + 1)
                npairs = nkt // 2
                gens = []
                n_items = 0
                if j + 1 < NJ:
                    qtiles[j + 1] = qtcp.tile(
                        [128, NE, 512], MM, tag="qt", name=f"qt{j+1}"
                    )
                    for et in range(NE):
                        gens.append(k_proj_group(et, j + 1))
                    for lt in range(4 * (j + 1), 4 * (j + 1) + 4):
                        gens.append(v_proj_group(lt))
                    for et in range(NE):
                        gens.append(q_proj_group(j + 1, qtiles[j + 1], et))
                    n_items += 12 * 9
                if j >= 1:
                    for lt in range(4 * (j - 1), 4 * (j - 1) + 4):
                        for ec in range(2):
                            gens.append(out_proj_group(lt, ec))
                    n_items += 8 * 6
                filler = chain(gens)
                steps = 8 * npairs
                per_step = n_items / steps if steps else 0.0
                QTc = qtiles[j]
                fill_acc = 0.0
                fill_done = 0

                for h in (1, 3, 5, 7, 0, 2, 4, 6):
                    hb, hp = h >> 1, (h & 1) * 64
                    PTt = ptp.tile([128, NK, 512], MM, tag="pt", name="pt")
                    avT = psA.tile([128, 512], F32, tag="av", name="avT")
                    for p in range(npairs):
                        sps = psS.tile([128, 2, 512], F32, tag="s", name="sps")
                        for half in (0, 1):
                            kt = 2 * p + half
                            r = kt - 4 * j
                            c0 = max(0, r * 128)
                            nc.tensor.matmul(
                                sps[:, half, c0:512],
                                KT[hp:hp + 64, hb, kt * 128:(kt + 1) * 128],
                                QTc[hp:hp + 64, hb, c0:512],
                            )
                        if 2 * p >= 4 * j:
                            # both k-tiles of this pair are diagonal: trim exp
                            for half in (0, 1):
                                kt = 2 * p + half
                                c0 = (kt - 4 * j) * 128
                                nc.scalar.activation(
                                    PTt[:, kt, c0:512], sps[:, half, c0:512], AF.Exp
                                )
                        else:
                            nc.scalar.activation(
                                PTt[:, 2 * p:2 * p + 2, :], sps[:], AF.Exp
                            )
                        for half in (0, 1):
                            kt = 2 * p + half
                            r = kt - 4 * j
                            if r >= 0:
                                nc.vector.tensor_mul(
                                    PTt[:, kt, r * 128:(r + 1) * 128],
                                    PTt[:, kt, r * 128:(r + 1) * 128],
                                    tri_sb[:],
                                )
                        if p == 0 and pending_norm is not None:
                            emit_norm(pending_norm)
                            pending_norm = None
                        if p > 0:
                            for kt in (2 * p - 2, 2 * p - 1):
                                r = kt - 4 * j
                                c0 = max(0, r * 128)
                                nc.tensor.matmul(
                                    avT[:, c0:512],
                                    Vaug[:, kt, h, :],
                                    PTt[:, kt, c0:512],
                                    start=(kt == 0),
                                    stop=(kt == nkt - 1),
                                )
                        fill_acc += per_step
                        take = int(fill_acc) - fill_done
                        if take > 0:
                            drain(filler, take)
                            fill_done += take
                    for kt in (nkt - 2, nkt - 1):
                        r = kt - 4 * j
                        c0 = max(0, r * 128)
                        nc.tensor.matmul(
                            avT[:, c0:512],
                            Vaug[:, kt, h, :],
                            PTt[:, kt, c0:512],
                            start=(kt == 0),
                            stop=(kt == nkt - 1),
                        )
                    # DVE: cross-base copy to SBUF + fast approx reciprocal
                    dcp = smallp.tile([64, 512], F32, tag="dc", name="dcp")
                    nc.vector.tensor_copy(dcp[:], avT[64:128, :])
                    rc = smallp.tile([64, 512], F32, tag="rca", name="rca")
                    nc.vector.reciprocal_approx_fast(rc[:], dcp[:])
                    pending_norm = (avT, rc, h, j)
                if j == NJ - 1 and pending_norm is not None:
                    emit_norm(pending_norm)
                    pending_norm = None
                drain(filler, 10 ** 9)
                if pending_norm is not None:
                    emit_norm(pending_norm)
                    pending_norm = None

            for lt in range(4 * (NJ - 1), 4 * (NJ - 1) + 4):
                for ec in range(2):
                    drain(out_proj_group(lt, ec), 99)

    nc.compile()
    return nc


def build_tri():
    kp = np.arange(128)[:, None]
    qf = np.arange(128)[None, :]
    return (qf >= kp).astype(np.float16)


def _get_program():
    if "nc" not in _CACHE:
        _CACHE["nc"] = build_program()
    return _CACHE["nc"]


def make_in_maps(x, Wq, Wk, Wv, Wo):
    x = np.asarray(x, dtype=np.float32)
    Wq = np.asarray(Wq, dtype=np.float32)
    Wk = np.asarray(Wk, dtype=np.float32)
    Wv = np.asarray(Wv, dtype=np.float32)
    Wo = np.asarray(Wo, dtype=np.float32)
    tri = build_tri()
    xTs = [np.ascontiguousarray(x[b].T).astype(np.float16) for b in range(B)]
    in_maps = []
    for c in range(8):
        b, hg = c // 2, c % 2
        sl = slice(hg * E, (hg + 1) * E)
        in_maps.append(
            {
                "xT": xTs[b],
                "wqT": np.ascontiguousarray((Wq[sl] * 0.125).T).astype(np.float16),
                "wkT": np.ascontiguousarray(Wk[sl].T).astype(np.float16),
                "wvT": np.ascontiguousarray(Wv[sl].T).astype(np.float16),
                "woT": np.ascontiguousarray(Wo[:, sl].T).astype(np.float16),
                "tri": tri,
            }
        )
    return in_maps


def kernel(x, Wq, Wk, Wv, Wo, **run_kwargs):
    from concourse import bass_utils

    nc = _get_program()
    in_maps = make_in_maps(x, Wq, Wk, Wv, Wo)
    res = bass_utils.run_bass_kernel_spmd(
        nc, in_maps, core_ids=list(range(8)), **run_kwargs
    )
    o = np.empty((B, L, D), np.float32)
    for b in range(B):
        o[b] = res.results[2 * b]["out"].astype(np.float32) + res.results[
            2 * b + 1
        ]["out"].astype(np.float32)
    _CACHE["last_result"] = res
    return o
